# revision 7
# baseline (speedup 1.0000x reference)
"""EntmaxBisectLoss (alpha=1.5) Trainium2 kernel, 8-core data-parallel.

Math: with alpha=1.5, p_unnorm = relu(0.5*X - tau)^2.  tau solves
f(tau) = sum_j relu(Xs_j - tau)^2 = 1  (Xs = 0.5*X).  Instead of 50
bisection passes we run Newton on g = sqrt(f): g is convex & decreasing
(Cauchy-Schwarz), so from the left it converges monotonically:
    delta = (S2 - sqrt(S2)) / S1,   S1 = sum u, S2 = sum u^2.
The loss needs only per-row scalars (u = relu(Xs - tau)):
    Z = sum u^2,  P3 = sum u^3
    sum p^1.5 = P3/Z^1.5,   sum p_j X_j = 2*(P3 + tau*Z)/Z
    loss_i = (1 - P3*Z^-1.5)/0.75 + 2*P3/Z + 2*tau - X[i, target_i]
and the loss is second-order insensitive to tau error at the optimum
(tau err 5e-3 -> loss rel err ~1e-5), so tau to the bf16 grid is enough.

Device state per 128-row block: V = max(Xs, tau) kept in SBUF as bf16
(clamp-form; elements at the clamp hold exactly bf16(tau), so evaluating
with bias = -bf16(tau) zeroes them exactly -> tau is quantized to the
bf16 grid each eval).  Per eval:
  update : V <- V max taub        (DVE tensor_scalar, accum = sum V;
                                   S1 = accum - N*taub via tiny ops)
  squares: ACT activation(Square, bias=-taub, accum) -> sum u^2 directly;
           DVE chunks use scalar_tensor_tensor (V-taub)*V with accum
           (= S2 + taub*S1_chunk, corrected via tiny ops)
Final Z/P3 from V: P3 chunk = STT (V-taub)*W with W = ACT square output.
Host: gather X[i,target], assemble loss, mean.  HBM traffic: 2 streams
(max pass, form pass); everything else runs out of SBUF.
"""

import numpy as np

ALPHA = 1.5
IGNORE_INDEX = -100
ROWS, COLS = 4096, 32000
N_CORES = 8
RS = ROWS // N_CORES          # 512 rows per core
P = 128                       # SBUF partitions
F_STREAM = 2000               # stream-chunk columns (1 MiB DMAs)
F_SQ = 4000                   # square/update chunk columns
K_EVALS = 4                   # sqrt-Newton evals
SQ_DVE = 2                    # square chunks (of COLS//F_SQ) on DVE, evals >=1
E0_DVE = 0                    # square chunks on DVE for eval 0

_NC_CACHE = {}


def _build_nc(rs=RS, cols=COLS, f_stream=F_STREAM, f_sq=F_SQ,
              k_evals=K_EVALS, sq_dve=SQ_DVE, e0_dve=E0_DVE):
    from concourse import bacc, mybir, tile

    f32 = mybir.dt.float32
    bf16 = mybir.dt.bfloat16
    AX = mybir.AxisListType.X
    OP = mybir.AluOpType
    ACTF = mybir.ActivationFunctionType

    nblk = rs // P
    nch_s = cols // f_stream
    nch_q = cols // f_sq
    ratio = f_sq // f_stream

    nc = bacc.Bacc(None, target_bir_lowering=False)
    xs_ext = nc.declare_dram_parameter("Xs", [rs, cols], f32, isOutput=False)
    out_ext = nc.declare_dram_parameter("out", [rs, 4], f32, isOutput=True)

    with tile.TileContext(nc) as tc:
        with (
            tc.tile_pool(name="u", bufs=2) as u_pool,
            tc.tile_pool(name="xch", bufs=4) as xch_pool,
            tc.tile_pool(name="w", bufs=2) as w_pool,
            tc.tile_pool(name="st", bufs=3) as st_pool,
        ):
            for b in range(nblk):
                rows = slice(b * P, (b + 1) * P)

                def tiny(tag):
                    return st_pool.tile([P, 1], f32, tag=tag, name=tag)

                def quantize(tau_in):
                    """Round tau to the bf16 grid; return (taub, negtaub)."""
                    tb16 = st_pool.tile([P, 1], bf16, tag="tb16")
                    nc.vector.tensor_copy(tb16[:], tau_in[:])
                    taub = tiny("taub")
                    nc.vector.tensor_copy(taub[:], tb16[:])
                    negtaub = tiny("negtaub")
                    nc.vector.tensor_scalar(
                        out=negtaub[:], in0=taub[:], scalar1=-1.0,
                        scalar2=None, op0=OP.mult)
                    return taub, negtaub

                def fold_s1(s1p_tile, negtaub):
                    """S1 = sum(accums) + negtaub*cols."""
                    s1raw = tiny("s1raw")
                    nc.vector.tensor_reduce(
                        out=s1raw[:], in_=s1p_tile[:], axis=AX, op=OP.add)
                    t1 = tiny("t1")
                    nc.vector.tensor_scalar(
                        out=t1[:], in0=negtaub[:], scalar1=float(cols),
                        scalar2=None, op0=OP.mult)
                    s1t = tiny("s1t")
                    nc.vector.tensor_add(s1t[:], s1raw[:], t1[:])
                    return s1t

                # ---- pass 1: row max over streamed chunks
                mxp = st_pool.tile([P, nch_s], f32, tag="mxp")
                for c in range(nch_s):
                    xch = xch_pool.tile([P, f_stream], f32, tag="xch")
                    nc.sync.dma_start(
                        out=xch[:],
                        in_=xs_ext[rows, c * f_stream:(c + 1) * f_stream])
                    nc.vector.tensor_reduce(
                        out=mxp[:, c:c + 1], in_=xch[:], axis=AX, op=OP.max)
                m = tiny("m")
                nc.vector.tensor_reduce(out=m[:], in_=mxp[:], axis=AX, op=OP.max)
                tau0 = tiny("tau")
                nc.vector.tensor_scalar(
                    out=tau0[:], in0=m[:], scalar1=1.0, scalar2=None,
                    op0=OP.subtract)
                taub, negtaub = quantize(tau0)

                # ---- pass 2: form V = max(Xs, taub), accum = sum V per chunk
                V = u_pool.tile([P, cols], bf16, tag="V")
                s1p = st_pool.tile([P, nch_s], f32, tag="s1p")
                for c in range(nch_s):
                    xch = xch_pool.tile([P, f_stream], f32, tag="xch")
                    nc.sync.dma_start(
                        out=xch[:],
                        in_=xs_ext[rows, c * f_stream:(c + 1) * f_stream])
                    nc.vector.tensor_scalar(
                        out=V[:, c * f_stream:(c + 1) * f_stream],
                        in0=xch[:], scalar1=taub[:], scalar2=None,
                        op0=OP.max, op1=OP.add,
                        accum_out=s1p[:, c:c + 1])
                s1t = fold_s1(s1p, negtaub)

                # ---- Newton evals (all sqrt-Newton)
                for j in range(k_evals):
                    if j > 0:
                        s1p = st_pool.tile([P, nch_q], f32, tag="s1p")
                        for c in range(nch_q):
                            cs = slice(c * f_sq, (c + 1) * f_sq)
                            nc.vector.tensor_scalar(
                                out=V[:, cs], in0=V[:, cs],
                                scalar1=taub[:], scalar2=None,
                                op0=OP.max, op1=OP.add,
                                accum_out=s1p[:, c:c + 1])
                        s1t = fold_s1(s1p, negtaub)
                    # squares -> S2
                    a_j = e0_dve if j == 0 else sq_dve
                    s2p = st_pool.tile([P, nch_q], f32, tag="s2p")
                    for c in range(nch_q):
                        cs = slice(c * f_sq, (c + 1) * f_sq)
                        w = w_pool.tile([P, f_sq], bf16, tag="w")
                        if c < a_j:
                            nc.vector.scalar_tensor_tensor(
                                out=w[:], in0=V[:, cs], scalar=negtaub[:],
                                in1=V[:, cs], op0=OP.add, op1=OP.mult,
                                accum_out=s2p[:, c:c + 1])
                        else:
                            nc.scalar.activation(
                                out=w[:], in_=V[:, cs], func=ACTF.Square,
                                bias=negtaub[:], scale=1.0,
                                accum_out=s2p[:, c:c + 1])
                    s2raw = tiny("s2raw")
                    nc.vector.tensor_reduce(
                        out=s2raw[:], in_=s2p[:], axis=AX, op=OP.add)
                    if a_j > 0:
                        # DVE chunks summed (V-taub)*V = u^2 + taub*u:
                        # S2 += negtaub * (sum of u over those chunks)
                        pre = a_j * (ratio if j == 0 else 1)
                        s1d_raw = tiny("s1d")
                        nc.vector.tensor_reduce(
                            out=s1d_raw[:], in_=s1p[:, :pre], axis=AX, op=OP.add)
                        # chunk u-sums: accum_c = sum(V) over chunk;
                        # u-sum_c = accum_c + negtaub*chunk_cols
                        ncols = pre * (f_stream if j == 0 else f_sq)
                        t2 = tiny("t2")
                        nc.vector.tensor_scalar(
                            out=t2[:], in0=negtaub[:], scalar1=float(ncols),
                            scalar2=None, op0=OP.mult)
                        s1d = tiny("s1d2")
                        nc.vector.tensor_add(s1d[:], s1d_raw[:], t2[:])
                        corr = tiny("corr")
                        nc.vector.tensor_mul(corr[:], negtaub[:], s1d[:])
                        s2t = tiny("s2t")
                        nc.vector.tensor_add(s2t[:], s2raw[:], corr[:])
                    else:
                        s2t = s2raw
                    # delta = (S2 - sqrt(S2)) / S1
                    inv = tiny("inv")
                    nc.vector.reciprocal(out=inv[:], in_=s1t[:])
                    r = tiny("r")
                    nc.scalar.activation(out=r[:], in_=s2t[:], func=ACTF.Sqrt)
                    num = tiny("num")
                    nc.vector.tensor_sub(num[:], s2t[:], r[:])
                    delta = tiny("delta")
                    nc.vector.tensor_mul(delta[:], num[:], inv[:])
                    tau_n = tiny("tau")
                    nc.vector.tensor_add(tau_n[:], taub[:], delta[:])
                    taub, negtaub = quantize(tau_n)

                # ---- final: V-update @ final taub, then Z and P3
                for c in range(nch_q):
                    cs = slice(c * f_sq, (c + 1) * f_sq)
                    nc.vector.tensor_scalar(
                        out=V[:, cs], in0=V[:, cs], scalar1=taub[:],
                        scalar2=None, op0=OP.max)
                zp = st_pool.tile([P, nch_q], f32, tag="s2p")
                p3p = st_pool.tile([P, nch_q], f32, tag="p3p")
                for c in range(nch_q):
                    cs = slice(c * f_sq, (c + 1) * f_sq)
                    w = w_pool.tile([P, f_sq], bf16, tag="w")
                    nc.scalar.activation(
                        out=w[:], in_=V[:, cs], func=ACTF.Square,
                        bias=negtaub[:], scale=1.0,
                        accum_out=zp[:, c:c + 1])
                    w3 = w_pool.tile([P, f_sq], bf16, tag="w3")
                    nc.vector.scalar_tensor_tensor(
                        out=w3[:], in0=V[:, cs], scalar=negtaub[:],
                        in1=w[:], op0=OP.add, op1=OP.mult,
                        accum_out=p3p[:, c:c + 1])
                zt = tiny("zt")
                p3t = tiny("p3t")
                nc.vector.tensor_reduce(out=zt[:], in_=zp[:], axis=AX, op=OP.add)
                nc.vector.tensor_reduce(out=p3t[:], in_=p3p[:], axis=AX, op=OP.add)

                stats = st_pool.tile([P, 4], f32, tag="stats")
                nc.vector.tensor_copy(stats[:, 0:1], negtaub[:])
                nc.vector.tensor_copy(stats[:, 1:2], zt[:])
                nc.vector.tensor_copy(stats[:, 2:3], p3t[:])
                nc.vector.tensor_copy(stats[:, 3:4], s1t[:])
                nc.sync.dma_start(out=out_ext[rows, :], in_=stats[:])
    nc.finalize()
    return nc


def _get_nc(key="full", **kw):
    if key not in _NC_CACHE:
        _NC_CACHE[key] = _build_nc(**kw)
    return _NC_CACHE[key]


def _assemble_loss(X, target, stats):
    """Host glue: per-row loss from device stats + target gather + mean."""
    n = X.shape[0]
    negtau = stats[:, 0].astype(np.float64)
    Z = stats[:, 1].astype(np.float64)
    P3 = stats[:, 2].astype(np.float64)
    tau = -negtau
    valid = target != IGNORE_INDEX
    tgt = np.where(valid, target, 0).astype(np.int64)
    gather = X[np.arange(n), tgt].astype(np.float64)
    omega = (1.0 - P3 / Z ** 1.5) / (ALPHA * (ALPHA - 1.0))
    loss = omega + 2.0 * P3 / Z + 2.0 * tau - gather
    loss = np.where(valid, loss, 0.0)
    denom = max(int(valid.sum()), 1)
    return np.float32(loss.sum() / denom)


def _run_device(Xs, trace=False):
    """Run the SPMD kernel on 8 cores; returns (stats(4096,4), exec_time_ns)."""
    from concourse.bass_utils import run_bass_kernel_spmd

    nc = _get_nc()
    in_maps = [{"Xs": np.ascontiguousarray(Xs[i * RS:(i + 1) * RS])}
               for i in range(N_CORES)]
    out = run_bass_kernel_spmd(nc, in_maps, list(range(N_CORES)), trace=trace)
    stats = np.concatenate([out.results[i]["out"] for i in range(N_CORES)],
                           axis=0)
    return stats, out.exec_time_ns


def kernel(X, target):
    X = np.ascontiguousarray(np.asarray(X), dtype=np.float32)
    target = np.asarray(target)
    Xs = X * np.float32(0.5)
    stats, _ = _run_device(Xs)
    return _assemble_loss(X, target, stats)


# revision 11
# speedup vs baseline: 85.5030x; 85.5030x over previous
"""EntmaxBisectLoss (alpha=1.5) Trainium2 kernel, 8-core data-parallel.

Math: with alpha=1.5, p_unnorm = relu(0.5*X - tau)^2.  tau solves
f(tau) = sum_j relu(Xs_j - tau)^2 = 1  (Xs = 0.5*X).  Instead of 50
bisection passes we run Newton on g = sqrt(f): g is convex & decreasing
(Cauchy-Schwarz), so from the left it converges monotonically:
    delta = (S2 - sqrt(S2)) / S1,   S1 = sum u, S2 = sum u^2.
The loss needs only per-row scalars (u = relu(Xs - tau)):
    Z = sum u^2,  P3 = sum u^3
    sum p^1.5 = P3/Z^1.5,   sum p_j X_j = 2*(P3 + tau*Z)/Z
    loss_i = (1 - P3*Z^-1.5)/0.75 + 2*P3/Z + 2*tau - X[i, target_i]
and the loss is second-order insensitive to tau error at the optimum
(tau err 5e-3 -> loss rel err ~1e-5), so tau to the bf16 grid is enough.

Device state per 128-row block: V = max(Xs, tau) kept in SBUF as bf16
(clamp-form; elements at the clamp hold exactly bf16(tau), so evaluating
with bias = -bf16(tau) zeroes them exactly -> tau is quantized to the
bf16 grid each eval).  One HBM stream only: the form pass writes V = Xs
(max with -1e9) while its accum (reduce-op max) yields per-chunk row
maxes; a fixup update V <- V max tau0 then restores the clamp invariant.
Per eval:
  update : V <- V max taub        (DVE tensor_scalar, accum = sum V;
                                   S1 = accum - N*taub via tiny ops)
  squares: ACT activation(Square, bias=-taub, accum) -> sum u^2 directly;
           DVE chunks use scalar_tensor_tensor (V-taub)*V with accum
           (= S2 + taub*S1_chunk, corrected via tiny ops)
Final Z/P3 from V: P3 chunk = STT (V-taub)*W with W = ACT square output.
Host: gather X[i,target], assemble loss, mean.
"""

import numpy as np

ALPHA = 1.5
IGNORE_INDEX = -100
ROWS, COLS = 4096, 32000
N_CORES = 8
RS = ROWS // N_CORES          # 512 rows per core
P = 128                       # SBUF partitions
F_SQ = 4000                   # chunk columns (2 MiB DMAs on the form pass)
K_EVALS = 4                   # sqrt-Newton evals
SQ_DVE = 1                    # square chunks (of COLS//F_SQ) on DVE per eval
FIN_DVE = 0                   # final-Z square chunks on DVE

_NC_CACHE = {}


def _build_nc(rs=RS, cols=COLS, f_sq=F_SQ, k_evals=K_EVALS,
              sq_dve=SQ_DVE, fin_dve=FIN_DVE, reps=1):
    from concourse import bacc, mybir, tile

    f32 = mybir.dt.float32
    bf16 = mybir.dt.bfloat16
    AX = mybir.AxisListType.X
    OP = mybir.AluOpType
    ACTF = mybir.ActivationFunctionType

    nblk = rs // P
    nch = cols // f_sq

    nc = bacc.Bacc(None, target_bir_lowering=False)
    xs_ext = nc.declare_dram_parameter("Xs", [rs, cols], f32, isOutput=False)
    out_ext = nc.declare_dram_parameter("out", [rs, 4], f32, isOutput=True)

    with tile.TileContext(nc) as tc:
        with (
            tc.tile_pool(name="u", bufs=2) as u_pool,
            tc.tile_pool(name="xch", bufs=3) as xch_pool,
            tc.tile_pool(name="w", bufs=2) as w_pool,
            tc.tile_pool(name="st", bufs=3) as st_pool,
        ):
          for _rep in range(reps):
            for b in range(nblk):
                rows = slice(b * P, (b + 1) * P)

                def tiny(tag):
                    return st_pool.tile([P, 1], f32, tag=tag, name=tag)

                def quantize(tau_in):
                    """Round tau to the bf16 grid; return (taub, negtaub)."""
                    tb16 = st_pool.tile([P, 1], bf16, tag="tb16", name="tb16")
                    nc.vector.tensor_copy(tb16[:], tau_in[:])
                    taub = tiny("taub")
                    nc.vector.tensor_copy(taub[:], tb16[:])
                    negtaub = tiny("negtaub")
                    nc.vector.tensor_scalar(
                        out=negtaub[:], in0=taub[:], scalar1=-1.0,
                        scalar2=None, op0=OP.mult)
                    return taub, negtaub

                def update_v(V, taub, negtaub):
                    """V <- V max taub (accum sum); returns S1 [P,1]."""
                    s1p = st_pool.tile([P, nch], f32, tag="s1p", name="s1p")
                    for c in range(nch):
                        cs = slice(c * f_sq, (c + 1) * f_sq)
                        nc.vector.tensor_scalar(
                            out=V[:, cs], in0=V[:, cs],
                            scalar1=taub[:], scalar2=None,
                            op0=OP.max, op1=OP.add,
                            accum_out=s1p[:, c:c + 1])
                    s1raw = tiny("s1raw")
                    nc.vector.tensor_reduce(
                        out=s1raw[:], in_=s1p[:], axis=AX, op=OP.add)
                    t1 = tiny("t1")
                    nc.vector.tensor_scalar(
                        out=t1[:], in0=negtaub[:], scalar1=float(cols),
                        scalar2=None, op0=OP.mult)
                    s1t = tiny("s1t")
                    nc.vector.tensor_add(s1t[:], s1raw[:], t1[:])
                    return s1t, s1p

                # ---- single stream: V = Xs (bf16), accum(max) = chunk maxes
                V = u_pool.tile([P, cols], bf16, tag="V", name="V")
                mxp = st_pool.tile([P, nch], f32, tag="mxp", name="mxp")
                for c in range(nch):
                    xch = xch_pool.tile([P, f_sq], f32, tag="xch", name="xch")
                    nc.sync.dma_start(
                        out=xch[:],
                        in_=xs_ext[rows, c * f_sq:(c + 1) * f_sq])
                    nc.vector.tensor_scalar(
                        out=V[:, c * f_sq:(c + 1) * f_sq],
                        in0=xch[:], scalar1=-1e9, scalar2=None,
                        op0=OP.max, op1=OP.max,
                        accum_out=mxp[:, c:c + 1])
                m = tiny("m")
                nc.vector.tensor_reduce(out=m[:], in_=mxp[:], axis=AX, op=OP.max)
                tau0 = tiny("tau")
                nc.vector.tensor_scalar(
                    out=tau0[:], in0=m[:], scalar1=1.0, scalar2=None,
                    op0=OP.subtract)
                taub, negtaub = quantize(tau0)

                # ---- Newton evals: update (fixup on eval 0) + squares.
                # Last eval reports tau/S2 directly (one Newton step behind
                # the unreported next tau — loss is 2nd-order insensitive)
                # and interleaves the P3 pass with its squares: per chunk,
                # ACT writes W = u^2 (accum S2) and DVE STT folds
                # (V-taub)*W = u^3 into P3.
                p3p = None
                for j in range(k_evals):
                    last = j == k_evals - 1
                    s1t, s1p = update_v(V, taub, negtaub)
                    ndve = 0 if last else sq_dve
                    s2p = st_pool.tile([P, nch], f32, tag="s2p", name="s2p")
                    if last:
                        p3p = st_pool.tile([P, nch], f32, tag="p3p", name="p3p")
                    for c in range(nch):
                        cs = slice(c * f_sq, (c + 1) * f_sq)
                        w = w_pool.tile([P, f_sq], bf16, tag="w", name="w")
                        if c < ndve:
                            nc.vector.scalar_tensor_tensor(
                                out=w[:], in0=V[:, cs], scalar=negtaub[:],
                                in1=V[:, cs], op0=OP.add, op1=OP.mult,
                                accum_out=s2p[:, c:c + 1])
                        else:
                            nc.scalar.activation(
                                out=w[:], in_=V[:, cs], func=ACTF.Square,
                                bias=negtaub[:], scale=1.0,
                                accum_out=s2p[:, c:c + 1])
                        if last:
                            w3 = w_pool.tile([P, f_sq], bf16, tag="w3",
                                             name="w3")
                            nc.vector.scalar_tensor_tensor(
                                out=w3[:], in0=V[:, cs], scalar=negtaub[:],
                                in1=w[:], op0=OP.add, op1=OP.mult,
                                accum_out=p3p[:, c:c + 1])
                    s2raw = tiny("s2raw")
                    nc.vector.tensor_reduce(
                        out=s2raw[:], in_=s2p[:], axis=AX, op=OP.add)
                    if ndve > 0:
                        # DVE chunks summed (V-taub)*V = u^2 + taub*u:
                        # S2 += negtaub * (u-sum over those chunks)
                        s1d_raw = tiny("s1d")
                        nc.vector.tensor_reduce(
                            out=s1d_raw[:], in_=s1p[:, :ndve], axis=AX,
                            op=OP.add)
                        t2 = tiny("t2")
                        nc.vector.tensor_scalar(
                            out=t2[:], in0=negtaub[:],
                            scalar1=float(ndve * f_sq),
                            scalar2=None, op0=OP.mult)
                        s1d = tiny("s1d2")
                        nc.vector.tensor_add(s1d[:], s1d_raw[:], t2[:])
                        corr = tiny("corr")
                        nc.vector.tensor_mul(corr[:], negtaub[:], s1d[:])
                        s2t = tiny("s2t")
                        nc.vector.tensor_add(s2t[:], s2raw[:], corr[:])
                    else:
                        s2t = s2raw
                    if not last:
                        # delta = (S2 - sqrt(S2)) / S1
                        inv = tiny("inv")
                        nc.vector.reciprocal(out=inv[:], in_=s1t[:])
                        r = tiny("r")
                        nc.scalar.activation(out=r[:], in_=s2t[:],
                                             func=ACTF.Sqrt)
                        num = tiny("num")
                        nc.vector.tensor_sub(num[:], s2t[:], r[:])
                        delta = tiny("delta")
                        nc.vector.tensor_mul(delta[:], num[:], inv[:])
                        tau_n = tiny("tau")
                        nc.vector.tensor_add(tau_n[:], taub[:], delta[:])
                        taub, negtaub = quantize(tau_n)

                zt = s2t
                p3t = tiny("p3t")
                nc.vector.tensor_reduce(out=p3t[:], in_=p3p[:], axis=AX, op=OP.add)

                stats = st_pool.tile([P, 4], f32, tag="stats", name="stats")
                nc.vector.tensor_copy(stats[:, 0:1], negtaub[:])
                nc.vector.tensor_copy(stats[:, 1:2], zt[:])
                nc.vector.tensor_copy(stats[:, 2:3], p3t[:])
                nc.vector.tensor_copy(stats[:, 3:4], s1t[:])
                nc.sync.dma_start(out=out_ext[rows, :], in_=stats[:])
    nc.finalize()
    return nc


def _get_nc(key="full", **kw):
    if key not in _NC_CACHE:
        _NC_CACHE[key] = _build_nc(**kw)
    return _NC_CACHE[key]


def _assemble_loss(X, target, stats):
    """Host glue: per-row loss from device stats + target gather + mean."""
    n = X.shape[0]
    negtau = stats[:, 0].astype(np.float64)
    Z = stats[:, 1].astype(np.float64)
    P3 = stats[:, 2].astype(np.float64)
    tau = -negtau
    valid = target != IGNORE_INDEX
    tgt = np.where(valid, target, 0).astype(np.int64)
    gather = X[np.arange(n), tgt].astype(np.float64)
    omega = (1.0 - P3 / Z ** 1.5) / (ALPHA * (ALPHA - 1.0))
    loss = omega + 2.0 * P3 / Z + 2.0 * tau - gather
    loss = np.where(valid, loss, 0.0)
    denom = max(int(valid.sum()), 1)
    return np.float32(loss.sum() / denom)


def _run_device(Xs, trace=False):
    """Run the SPMD kernel on 8 cores; returns (stats(4096,4), exec_time_ns)."""
    from concourse.bass_utils import run_bass_kernel_spmd

    nc = _get_nc()
    in_maps = [{"Xs": np.ascontiguousarray(Xs[i * RS:(i + 1) * RS])}
               for i in range(N_CORES)]
    out = run_bass_kernel_spmd(nc, in_maps, list(range(N_CORES)), trace=trace)
    stats = np.concatenate([out.results[i]["out"] for i in range(N_CORES)],
                           axis=0)
    return stats, out.exec_time_ns


def kernel(X, target):
    X = np.ascontiguousarray(np.asarray(X), dtype=np.float32)
    target = np.asarray(target)
    Xs = X * np.float32(0.5)
    stats, _ = _run_device(Xs)
    return _assemble_loss(X, target, stats)
